# revision 29
# baseline (speedup 1.0000x reference)
"""GQA attention kernel for Trainium2, 8 NeuronCores.

Sharding: core = b*4 + g  (b = batch 0..1, g = kv-head group 0..3).
Each core handles one batch and one kv group (1 kv head + its 4 query heads).
wq/wo are split by head group (column/row), wk/wv by kv head. The output
projection partial sums (one per group) are reduced on the host.

The four projections (Q/K/V and wo) run on the PE in fp8e4 DoubleRow mode
(0.5 cycles/row, 256-deep contraction per instruction) using a 3-term
hi/lo split:  a@b ~= ah@bh + ah@bl + al@bh,  where ah = e4m3(a*s) and
al = e4m3(a*s - ah) with a power-of-2 pre-scale s chosen on the host so
values sit in e4m3's normal range (weights sigma=0.02 are denormal in raw
e4m3).  That is 0.75x the bf16 cycle cost with ~0.4% relative error.
De-scales are folded into existing constants so no extra ops run:
  Q,K: cs/sn rope tables carry 2^-14        (x*16 @ w*1024)
  V:   stays scaled; the ones column of V holds 2^14/SO so the softmax
       normalization yields O*SO directly   (SO = 16)
  y:   partial outputs leave the core scaled by 2^14; host divides.
x and the weights are split hi/lo on the host (same DMA bytes as bf16).
O^T is split on-device: Act casts the transposed tile to fp8 (oh) and DVE
subtracts for the residual (ol).

Attention proper (scores, exp, P@V) stays bf16: P comes out of exp on the
Act engine and re-splitting 8.4M elements per core is not affordable, and
a plain-e4m3 P or Q/K costs ~3% output error (budget is 2e-2 with ~6e-3
used by bf16).  Layouts as in the bf16 baseline:
  S^T    = K_roped^T.T @ Q_roped^T          -> [keys, queries]
  softmax: exp on the Act engine (no max subtraction; scores are O(1)),
           two key-tiles per exp instruction to amortize access latency.
  P@V    : P^T subtile is the STATIONARY operand, V|ones the moving one,
           so out = [q, d+1] and the softmax denominator rides in column
           d for free (no separate ones-matmul).
  O^T    : per-partition reciprocal * scale, then PE transposes back to
           [d, q] for the wo projection.
RoPE in [d, s] layout: rope(Z) = Z*C + (Pswap @ Z)*Sg, pair-swap done on
the PE, sign folded into the host-built Sg tile.

The PE is kept fed while the Act engine runs exp by draining a FIFO of
deferred PE work (P@V of the previous head, wo of the previous chunk)
between score matmuls; each FIFO entry carries a PE-time estimate and
each score unit drains roughly its own Act-minus-PE deficit.
"""

import sys

sys.path.insert(0, "/opt/trn_rl_repo")

from collections import deque
from contextlib import ExitStack

import numpy as np
import ml_dtypes

import concourse.bass as bass
import concourse.tile as tile
from concourse import bacc, mybir
from concourse import bass_utils

F32 = mybir.dt.float32
BF = mybir.dt.bfloat16
FP8 = mybir.dt.float8e4
DR = mybir.MatmulPerfMode.DoubleRow
MULT = mybir.AluOpType.mult
SUB = mybir.AluOpType.subtract
EXP = mybir.ActivationFunctionType.Exp

S = 2048          # sequence length
DM = 2048         # d_model
DH = 128          # head dim
HPC = 4           # query heads per core (= n_rep; one kv group per core)
N_CORES = 8
CH = 512          # query-chunk width (and s-chunk width)
NCHUNK = S // CH  # 4
NT = DM // 128    # 16 contraction tiles of d_model
NP = NT // 2      # 8 DoubleRow pairs of d_model
SCALE = 1.0 / float(np.sqrt(DH))
NPBF = ml_dtypes.bfloat16
NPFP8 = ml_dtypes.float8_e4m3

SX = 16.0         # x pre-scale (2^4)
SW = 1024.0       # weight pre-scale (2^10)
SO = 16.0         # O pre-scale for the wo projection (2^4)
DRAIN_FULL = 1000  # drain budget after a full-pair score unit
DRAIN_D1 = 900    # after the first diagonal unit
DRAIN_D2 = 700    # after the second diagonal unit
DRAIN_K = 800     # after the K-projection matmuls
DRAIN_V = 700     # after the V-projection matmuls
CS_ON_SCALAR = False  # cs/sn DMA on the scalar queue (vs gpsimd)
OH_ON_DVE = False     # fp8 hi cast of O^T on DVE (vs Act)
ROPE_RAW_ACT = False  # alternate rope raw copies onto Act
WO_OL_LAST = True     # wo matmul term order: ol-dependent last
FINE_DMA = False      # 2-tile startup DMA pieces
XPREF_SCALAR = False  # x prefetch rides the scalar queue behind the weights
DEFER_ROPE1 = False   # defer chunk-0 q1 rope past the K matmuls
DRAIN_LAST = 1.0      # drain-budget multiplier on the last chunk
DEFER_WO_DTS = 0      # wo dts of chunk 1 deferred into chunk 3's drain pool
OH_DVE_LAST = False   # chunk-3 oh casts on DVE (Act is the binding engine)
O_DMA_TR = False      # O^T via DMA XBAR for heads 0..2
ONES_VAL = (SX * SW) / SO        # V stays scaled by 2^14; see docstring
Y_DESCALE = 1.0 / (SO * SW)      # host-side

_CACHE = {}


def _build():
    nc = bacc.Bacc("TRN2", target_bir_lowering=False, debug=False)

    xh = nc.dram_tensor("xh", [DM, S], FP8, kind="ExternalInput").ap()
    xl = nc.dram_tensor("xl", [DM, S], FP8, kind="ExternalInput").ap()
    wqh = nc.dram_tensor("wqh", [DM, HPC * DH], FP8, kind="ExternalInput").ap()
    wql = nc.dram_tensor("wql", [DM, HPC * DH], FP8, kind="ExternalInput").ap()
    wkh = nc.dram_tensor("wkh", [DM, DH], FP8, kind="ExternalInput").ap()
    wkl = nc.dram_tensor("wkl", [DM, DH], FP8, kind="ExternalInput").ap()
    wvh = nc.dram_tensor("wvh", [DM, DH], FP8, kind="ExternalInput").ap()
    wvl = nc.dram_tensor("wvl", [DM, DH], FP8, kind="ExternalInput").ap()
    woh = nc.dram_tensor("woh", [HPC * DH, DM], FP8, kind="ExternalInput").ap()
    wol = nc.dram_tensor("wol", [HPC * DH, DM], FP8, kind="ExternalInput").ap()
    cs = nc.dram_tensor("cs", [DH, S], BF, kind="ExternalInput").ap()
    sn = nc.dram_tensor("sn", [DH, S], BF, kind="ExternalInput").ap()
    pswap = nc.dram_tensor("pswap", [DH, DH], BF, kind="ExternalInput").ap()
    ident = nc.dram_tensor("ident", [128, 128], BF, kind="ExternalInput").ap()
    maskd = nc.dram_tensor("maskd", [128, 128], BF, kind="ExternalInput").ap()
    yT = nc.dram_tensor("yT", [DM, S], BF, kind="ExternalOutput").ap()

    with tile.TileContext(nc) as tc, ExitStack() as ctx:
        consts = ctx.enter_context(tc.tile_pool(name="consts", bufs=1))
        wpool = ctx.enter_context(tc.tile_pool(name="wpool", bufs=1))
        persist = ctx.enter_context(tc.tile_pool(name="persist", bufs=1))
        xpool = ctx.enter_context(tc.tile_pool(name="xpool", bufs=2))
        qpool = ctx.enter_context(tc.tile_pool(name="qpool", bufs=2))
        rawp = ctx.enter_context(tc.tile_pool(name="rawp", bufs=4))
        ropet = ctx.enter_context(tc.tile_pool(name="ropet", bufs=4))
        ppool = ctx.enter_context(tc.tile_pool(name="ppool", bufs=16))
        vtp = ctx.enter_context(tc.tile_pool(name="vtp", bufs=2))
        osbp = ctx.enter_context(tc.tile_pool(name="osbp", bufs=3))
        rdp = ctx.enter_context(tc.tile_pool(name="rdp", bufs=6))
        otsb = ctx.enter_context(tc.tile_pool(name="otsb", bufs=2))
        ystp = ctx.enter_context(tc.tile_pool(name="ystp", bufs=2))
        # PSUM: 8 banks = acc 2x1 + st2 2x2 + ot0/ot1 1x1 each.  Every st2
        # tenant (score pairs, rope pswap, V/O transposes, wo accum) is
        # short-lived so the 2-slot ring never blocks on a long hold.
        ps_acc = ctx.enter_context(tc.tile_pool(name="ps_acc", bufs=2, space="PSUM"))
        ps_st2 = ctx.enter_context(tc.tile_pool(name="ps_st2", bufs=2, space="PSUM"))
        ps_ot = ctx.enter_context(tc.tile_pool(name="ps_ot", bufs=1, space="PSUM"))

        # ---------------- initial DMAs ----------------
        # x chunk 0: first DoubleRow pair of xh alone on the gpsimd queue so
        # the first Q matmul starts as early as possible, the rest of xh then
        # xl on the sync queue; weights on the scalar queue in consumption
        # order (wq hi, wq lo, wk, wv, wo); small constants via the Pool
        # SWDGE path.
        xh_r = xh.rearrange("(t p) n -> p t n", p=128)
        xl_r = xl.rearrange("(t p) n -> p t n", p=128)
        xc0h = xpool.tile([128, NT, CH], FP8, tag="xch")
        xc0l = xpool.tile([128, NT, CH], FP8, tag="xcl")
        wqh_r = wqh.rearrange("(t p) n -> p t n", p=128)
        wql_r = wql.rearrange("(t p) n -> p t n", p=128)
        wqh_sb = wpool.tile([128, NT, HPC * DH], FP8, tag="wqh")
        wql_sb = wpool.tile([128, NT, HPC * DH], FP8, tag="wql")
        nc.gpsimd.dma_start(xc0h[:, 0:2, :], xh_r[:, 0:2, 0:CH])
        nc.scalar.dma_start(wqh_sb[:, 0:2, :], wqh_r[:, 0:2, :])
        if FINE_DMA:
            xh_pieces = [(a, a + 2) for a in range(2, 16, 2)]
            xl_pieces = [(a, a + 2) for a in range(0, 16, 2)]
            wqh_pieces = [(a, a + 2) for a in range(2, 16, 2)]
            wql_pieces = [(a, a + 4) for a in range(0, 16, 4)]
        else:
            xh_pieces = [(2, 4), (4, 8), (8, 12), (12, 16)]
            xl_pieces = [(0, 4), (4, 10), (10, 16)]
            wqh_pieces = [(2, 6), (6, 11), (11, 16)]
            wql_pieces = [(0, 6), (6, 11), (11, 16)]
        for a, b in xh_pieces:
            nc.sync.dma_start(xc0h[:, a:b, :], xh_r[:, a:b, 0:CH])
        for a, b in xl_pieces:
            nc.sync.dma_start(xc0l[:, a:b, :], xl_r[:, a:b, 0:CH])
        for a, b in wqh_pieces:
            nc.scalar.dma_start(wqh_sb[:, a:b, :], wqh_r[:, a:b, :])
        for a, b in wql_pieces:
            nc.scalar.dma_start(wql_sb[:, a:b, :], wql_r[:, a:b, :])
        wkh_sb = wpool.tile([128, NT, DH], FP8, tag="wkh")
        nc.scalar.dma_start(wkh_sb, wkh.rearrange("(t p) n -> p t n", p=128))
        wkl_sb = wpool.tile([128, NT, DH], FP8, tag="wkl")
        nc.scalar.dma_start(wkl_sb, wkl.rearrange("(t p) n -> p t n", p=128))
        wvh_sb = wpool.tile([128, NT, DH], FP8, tag="wvh")
        nc.scalar.dma_start(wvh_sb, wvh.rearrange("(t p) n -> p t n", p=128))
        wvl_sb = wpool.tile([128, NT, DH], FP8, tag="wvl")
        nc.scalar.dma_start(wvl_sb, wvl.rearrange("(t p) n -> p t n", p=128))

        # cs/sn (1 MB) ride the scalar queue: on the gpsimd queue they starve
        # the chunk-0 x/wq stream of DMA-engine time in the first ~3us.  Only
        # chunk 0's columns are needed early (first rope ~t=18us); the rest
        # follows after the K/V weights.
        csq = nc.scalar if CS_ON_SCALAR else nc.gpsimd
        cs_sb = consts.tile([DH, S], BF, tag="cs")
        csq.dma_start(cs_sb[:, 0:CH], cs[:, 0:CH])
        sn_sb = consts.tile([DH, S], BF, tag="sn")
        csq.dma_start(sn_sb[:, 0:CH], sn[:, 0:CH])
        pswap_sb = consts.tile([DH, DH], BF, tag="pswap")
        csq.dma_start(pswap_sb, pswap)
        ident_sb = consts.tile([128, 128], BF, tag="ident")
        nc.gpsimd.dma_start(ident_sb, ident)
        maskd_sb = consts.tile([128, 128], BF, tag="maskd")
        nc.gpsimd.dma_start(maskd_sb, maskd)

        csq.dma_start(cs_sb[:, CH:], cs[:, CH:])
        csq.dma_start(sn_sb[:, CH:], sn[:, CH:])
        woh_sb = wpool.tile([128, HPC, DM], FP8, tag="woh")
        nc.scalar.dma_start(woh_sb, woh.rearrange("(h p) n -> p h n", p=128))
        wol_sb = wpool.tile([128, HPC, DM], FP8, tag="wol")
        nc.scalar.dma_start(wol_sb, wol.rearrange("(h p) n -> p h n", p=128))

        kt_sb = persist.tile([DH, S], BF, tag="kt")            # roped K^T
        # V in [s, d] layout (still scaled by SX*SW) + a normalization column
        # at d=128 holding SX*SW/SO (memset once; the transpose copies only
        # overwrite [:, j, :128] so col 128 keeps that value).
        v_sb = persist.tile([128, S // 128, 130], BF, tag="v")
        nc.gpsimd.memset(v_sb, ONES_VAL)

        yT_r = yT.rearrange("(t p) n -> p t n", p=128)

        def rope_start(raw_ps, raw_on_act=False):
            """Stage 1: PSUM -> SBUF copy of the raw projection.  Emitting
            other PE work between start and finish hides the copy latency
            (the PE is in-order, so the pswap matmul must come after filler).
            """
            raw_sb = rawp.tile([128, CH], BF, tag="raw")
            if raw_on_act and ROPE_RAW_ACT:
                nc.scalar.copy(raw_sb, raw_ps)
            else:
                nc.vector.tensor_copy(raw_sb, raw_ps)
            return raw_sb

        def rope_finish(raw_sb, c, out_ap):
            """Stage 2: out = raw*C + (Pswap @ raw)*Sg for s-chunk c.
            cs/sn carry the 2^-14 fp8 de-scale, so out is in true units."""
            col = c * CH
            sw_ps = ps_st2.tile([128, CH], F32, tag="st2")
            nc.tensor.matmul(sw_ps, pswap_sb, raw_sb, start=True, stop=True)
            ta = ropet.tile([128, CH], BF, tag="ra")
            nc.vector.tensor_tensor(ta, raw_sb, cs_sb[:, col:col + CH], MULT)
            tb = ropet.tile([128, CH], BF, tag="rb")
            nc.vector.tensor_tensor(tb, sw_ps, sn_sb[:, col:col + CH], MULT)
            nc.vector.tensor_add(out_ap, ta, tb)

        def rope(raw_ps, c, out_ap, raw_on_act=False):
            rope_finish(rope_start(raw_ps, raw_on_act), c, out_ap)

        # The 3 hi/lo split terms for a projection: (w_hi, x_hi), (w_hi,
        # x_lo), (w_lo, x_hi).  Term-major emission so chunk 0 can start on
        # the hi tensors while the lo DMAs are still in flight.
        def terms(wh_, wl_, xh_, xl_):
            return ((wh_, xh_), (wh_, xl_), (wl_, xh_))

        # FIFO of (pe_ns, thunk): deferred PE work, drained between score
        # units so the PE stays fed while Act runs exp.
        fifo = deque()
        markers_done = set()

        def drain(budget_ns):
            while fifo and budget_ns > 0:
                ns, thunk = fifo.popleft()
                thunk()
                budget_ns -= max(ns, 1)

        def drain_until(mid):
            """Pop until the marker `mid` has been consumed.  Guarantees the
            P@V consumers of two-heads-ago P tiles are emitted before their
            ring slots are re-allocated (else the scheduler deadlocks)."""
            if isinstance(mid, int) and mid < 0:
                return
            while mid not in markers_done:
                ns, thunk = fifo.popleft()
                thunk()

        xcs = {}
        qts = {}
        deferred_wo = []

        def push_qpass(cn, xcnh, xcnl):
            """Queue the Q projection + rope of chunk cn as PE filler for the
            current chunk's attention phase."""
            qt_n = qpool.tile([128, HPC, CH], BF, tag="qt", name=f"qt{cn}")
            qts[cn] = qt_n
            state = {}

            def mk_q_t(pair, term, p):
                def f():
                    if term == 0 and p == 0:
                        state[pair] = [
                            ps_acc.tile([128, CH], F32, tag="acc",
                                        name=f"accq{cn}_{pair}_{i}")
                            for i in range(2)]
                    wt, xt = terms(wqh_sb, wql_sb, xcnh, xcnl)[term]
                    for i in range(2):
                        h = 2 * pair + i
                        nc.tensor.matmul(
                            state[pair][i],
                            wt[:, 2 * p:2 * p + 2, h * DH:(h + 1) * DH],
                            xt[:, 2 * p:2 * p + 2, :],
                            start=(term == 0 and p == 0),
                            stop=(term == 2 and p == NP - 1),
                            perf_mode=DR, skip_group_check=True)
                return f

            def mk_q_rope(pair):
                def f():
                    for i in range(2):
                        rope(state[pair][i], cn, qt_n[:, 2 * pair + i, :])
                return f

            for pair in range(2):
                for term in range(3):
                    for p in range(NP):
                        fifo.append((214, mk_q_t(pair, term, p)))
                fifo.append((800, mk_q_rope(pair)))
            fifo.append((0, lambda mid=("q", cn): markers_done.add(mid)))

        for c in range(NCHUNK):
            col = c * CH

            # ------------ Q projection ------------
            # Chunk 0: inline single 4-head pass (h2/h3 accumulate in the
            # idle st2 slots) keeping the PE abreast of the initial x/wq DMA
            # stream.  Later chunks were queued as FIFO filler during the
            # previous chunk's attention -- just make sure they are emitted.
            if c == 0:
                xch, xcl = xc0h, xc0l
                qt_sb = qpool.tile([128, HPC, CH], BF, tag="qt", name="qt0")
                accs = [ps_acc.tile([128, CH], F32, tag="acc", name=f"accq0_{i}")
                        for i in range(2)]
                accs += [ps_st2.tile([128, CH], F32, tag="st2",
                                     name=f"accq0_{i + 2}") for i in range(2)]
                def q_mm(term, p, h):
                    wt, xt = terms(wqh_sb, wql_sb, xch, xcl)[term]
                    nc.tensor.matmul(
                        accs[h],
                        wt[:, 2 * p:2 * p + 2, h * DH:(h + 1) * DH],
                        xt[:, 2 * p:2 * p + 2, :],
                        start=(term == 0 and p == 0),
                        stop=(term == 2 and p == NP - 1),
                        perf_mode=DR)

                # terms 0/1 head-interleaved (DMA-paced); the final lh term
                # goes head-major so each head's rope chain starts while the
                # next head's matmuls keep the PE busy (st2 ring order forces
                # heads 2,3 first, and ropes 0/1 must free the ps_acc slots
                # before the K/V projections claim them).
                for term in (0, 1):
                    for p in range(NP):
                        for h in range(HPC):
                            q_mm(term, p, h)
                q_raws = {}
                rope_order = (2, 3, 0, 1)
                for idx, h in enumerate(rope_order):
                    for p in range(NP):
                        q_mm(2, p, h)
                    if idx > 0:
                        hprev = rope_order[idx - 1]
                        rope_finish(q_raws[hprev], c, qt_sb[:, hprev, :])
                    q_raws[h] = rope_start(accs[h])
                # q1's rope_finish is deferred past the K matmuls below
            else:
                xch, xcl = xcs[c]
                drain_until(("q", c))
                qt_sb = qts[c]

            # prefetch next chunk's x right away (transfers overlap K/V and
            # attention; the slot's previous readers finished last chunk)
            if c + 1 < NCHUNK:
                xnh = xpool.tile([128, NT, CH], FP8, tag="xch", name=f"xch{c + 1}")
                xnl = xpool.tile([128, NT, CH], FP8, tag="xcl", name=f"xcl{c + 1}")
                xcs[c + 1] = (xnh, xnl)
                ncol = (c + 1) * CH
                xq_ = nc.scalar if XPREF_SCALAR else nc.sync
                for i in range(4):
                    xq_.dma_start(
                        xnh[:, 4 * i:4 * i + 4, :],
                        xh_r[:, 4 * i:4 * i + 4, ncol:ncol + CH])
                for i in range(4):
                    xq_.dma_start(
                        xnl[:, 4 * i:4 * i + 4, :],
                        xl_r[:, 4 * i:4 * i + 4, ncol:ncol + CH])

            # ------------ K,V projections (K first: its rope chain is on
            # the critical path to the diagonal score tiles).  The K rope's
            # PSUM copy runs while the V matmuls keep the PE busy. ------------
            acc_k = ps_acc.tile([128, CH], F32, tag="acc")
            i = 0
            for wt, xt in terms(wkh_sb, wkl_sb, xch, xcl):
                for p in range(NP):
                    nc.tensor.matmul(acc_k, wt[:, 2 * p:2 * p + 2, :],
                                     xt[:, 2 * p:2 * p + 2, :],
                                     start=(i == 0), stop=(i == 3 * NP - 1),
                                     perf_mode=DR)
                    i += 1
            if c == 0:
                # chunk-0 q1 rope tail, deferred past the K matmuls
                rope_finish(q_raws[1], c, qt_sb[:, 1, :])
            else:
                drain(DRAIN_K)
            raw_k = rope_start(acc_k, raw_on_act=True)
            acc_v = ps_acc.tile([128, CH], F32, tag="acc")
            i = 0
            for wt, xt in terms(wvh_sb, wvl_sb, xch, xcl):
                for p in range(NP):
                    nc.tensor.matmul(acc_v, wt[:, 2 * p:2 * p + 2, :],
                                     xt[:, 2 * p:2 * p + 2, :],
                                     start=(i == 0), stop=(i == 3 * NP - 1),
                                     perf_mode=DR)
                    i += 1
            rope_finish(raw_k, c, kt_sb[:, col:col + CH])
            drain(DRAIN_V)   # fill the V PSUM->SBUF copy latency
            vt_sb = vtp.tile([128, CH], BF, tag="vt")
            nc.scalar.copy(vt_sb, acc_v)
            tpv = ps_st2.tile([128, CH], BF, tag="st2", name=f"tpv{c}")
            for u in range(4):
                nc.tensor.matmul(tpv[:, u * 128:(u + 1) * 128],
                                 vt_sb[:, u * 128:(u + 1) * 128], ident_sb,
                                 is_transpose=True, skip_group_check=True)
            nc.vector.tensor_copy(
                v_sb[:, 4 * c:4 * c + 4, :128],
                tpv.rearrange("p (u n) -> p u n", u=4))

            # queue next chunk's Q projection as attention-phase filler
            if c + 1 < NCHUNK:
                push_qpass(c + 1, *xcs[c + 1])

            # ------------ attention ------------
            def emit_scores(h):
                """Score+exp+mask for head h. Returns pslice: j -> (ap, base)
                so the P^T column block for (j, qsub u) is ap[:, base+128u:
                base+128(u+1)]."""
                pslice = {}
                # full key tiles, two per exp instruction
                for j in range(0, 4 * c, 2):
                    st2 = ps_st2.tile([128, 2 * CH], F32, tag="st2",
                                      name=f"st{c}_{h}_{j}")
                    nc.tensor.matmul(st2[:, :CH], kt_sb[:, j * 128:(j + 1) * 128],
                                     qt_sb[:, h, :], start=True, stop=True)
                    nc.tensor.matmul(st2[:, CH:], kt_sb[:, (j + 1) * 128:(j + 2) * 128],
                                     qt_sb[:, h, :], start=True, stop=True)
                    pp = ppool.tile([128, 2 * CH], BF, tag="pp",
                                    name=f"pp{c}_{h}_{j}")
                    nc.scalar.activation(pp, st2, EXP, scale=SCALE)
                    pslice[j] = (pp, 0)
                    pslice[j + 1] = (pp, CH)
                    drain(DRAIN_FULL * (DRAIN_LAST if c == NCHUNK - 1 else 1))
                # diagonal tiles: t=0,1 packed into [0:896], t=2,3 into
                # [0:384] (suffixes shifted left so the exp input is fully
                # written -- no stale-PSUM regions).
                dj = 4 * c
                st2 = ps_st2.tile([128, 896], F32, tag="st2",
                                  name=f"sd01_{c}_{h}")
                nc.tensor.matmul(st2[:, 0:CH], kt_sb[:, dj * 128:(dj + 1) * 128],
                                 qt_sb[:, h, :], start=True, stop=True)
                nc.tensor.matmul(st2[:, CH:896],
                                 kt_sb[:, (dj + 1) * 128:(dj + 2) * 128],
                                 qt_sb[:, h, 128:], start=True, stop=True)
                pp = ppool.tile([128, 896], BF, tag="pd", name=f"pd01_{c}_{h}",
                                bufs=4)
                nc.scalar.activation(pp, st2, EXP, scale=SCALE)
                nc.vector.tensor_tensor(pp[:, 0:128], pp[:, 0:128], maskd_sb, MULT)
                nc.vector.tensor_tensor(pp[:, CH:CH + 128], pp[:, CH:CH + 128],
                                        maskd_sb, MULT)
                pslice[dj] = (pp, 0)
                pslice[dj + 1] = (pp, CH - 128)   # col = 384 + 128u, u >= 1
                drain(DRAIN_D1 * (DRAIN_LAST if c == NCHUNK - 1 else 1))
                st2 = ps_st2.tile([128, 384], F32, tag="st2",
                                  name=f"sd23_{c}_{h}")
                nc.tensor.matmul(st2[:, 0:256],
                                 kt_sb[:, (dj + 2) * 128:(dj + 3) * 128],
                                 qt_sb[:, h, 256:], start=True, stop=True)
                nc.tensor.matmul(st2[:, 256:384],
                                 kt_sb[:, (dj + 3) * 128:(dj + 4) * 128],
                                 qt_sb[:, h, 384:], start=True, stop=True)
                pp = ppool.tile([128, 384], BF, tag="pd2", name=f"pd23_{c}_{h}",
                                bufs=4)
                nc.scalar.activation(pp, st2, EXP, scale=SCALE)
                nc.vector.tensor_tensor(pp[:, 0:128], pp[:, 0:128], maskd_sb, MULT)
                nc.vector.tensor_tensor(pp[:, 256:384], pp[:, 256:384],
                                        maskd_sb, MULT)
                pslice[dj + 2] = (pp, -256)       # col = -256 + 128u, u >= 2
                pslice[dj + 3] = (pp, -128)       # col = 256 at u == 3
                drain(DRAIN_D2 * (DRAIN_LAST if c == NCHUNK - 1 else 1))
                return pslice

            def push_pv(h, pslice, oh_sb, ol_sb):
                """Queue P@V + normalize + transpose + fp8 hi/lo split for
                head h."""
                ot0 = ps_ot.tile([128, 2, 132], F32, tag="ot0",
                                 name=f"ot0_{c}_{h}")
                ot1 = ps_ot.tile([128, 2, 132], F32, tag="ot1",
                                 name=f"ot1_{c}_{h}")
                osbs = []

                def mk_mm(u, j, first, last):
                    ot = ot0 if u < 2 else ot1
                    p, base = pslice[j]

                    def f():
                        nc.tensor.matmul(
                            ot[:, u % 2, :129],
                            p[:, base + u * 128:base + (u + 1) * 128],
                            v_sb[:, j, :129],
                            start=first, stop=last, skip_group_check=True)
                    return f

                def mk_div(u):
                    ot = ot0 if u < 2 else ot1

                    def f():
                        rd = rdp.tile([128, 1], F32, tag="rd")
                        nc.vector.reciprocal(rd, ot[:, u % 2, 128:129])
                        if u == 0:
                            osbs.append(osbp.tile([128, 4, 128], BF, tag="osb",
                                                  name=f"osb{c}_{h}"))
                        nc.vector.tensor_scalar_mul(osbs[0][:, u, :],
                                                    ot[:, u % 2, :128], rd)
                    return f

                def mk_fin(oh_sb=oh_sb, ol_sb=ol_sb):
                    # Heads 0..2: transpose O on the DMA XBAR (off the PE and
                    # out of the st2 ring; out[p,t,f] = in[f, 128t+p] matches
                    # the [d, q] layout).  Last head: keep the lower-latency
                    # PE transpose -- it gates the chunk's wo drains.
                    def f_dma():
                        ob = osbp.tile([128, 4, 128], BF, tag="ob", bufs=3,
                                       name=f"ob{c}_{h}")
                        nc.sync.dma_start_transpose(ob, osbs[0])
                        obf = ob.rearrange("p a b -> p (a b)")
                        if OH_ON_DVE or (OH_DVE_LAST and c == NCHUNK - 1):
                            nc.vector.tensor_copy(oh_sb[:, h, :], obf)
                        else:
                            nc.scalar.copy(oh_sb[:, h, :], obf)
                        nc.vector.tensor_tensor(ol_sb[:, h, :], obf,
                                                oh_sb[:, h, :], SUB)

                    def f_pe():
                        tp2 = ps_st2.tile([128, CH], BF, tag="st2",
                                          name=f"tp2_{c}_{h}")
                        for u in range(4):
                            nc.tensor.matmul(tp2[:, u * 128:(u + 1) * 128],
                                             osbs[0][:, u, :], ident_sb,
                                             is_transpose=True,
                                             skip_group_check=True)
                        if OH_ON_DVE or (OH_DVE_LAST and c == NCHUNK - 1):
                            nc.vector.tensor_copy(oh_sb[:, h, :], tp2)
                        else:
                            nc.scalar.copy(oh_sb[:, h, :], tp2)
                        nc.vector.tensor_tensor(ol_sb[:, h, :], tp2,
                                                oh_sb[:, h, :], SUB)
                    return f_pe if (h == HPC - 1 or not O_DMA_TR) else f_dma

                for u in range(4):
                    js = list(range(4 * c + u + 1))
                    for idx, j in enumerate(js):
                        fifo.append(
                            (54, mk_mm(u, j, idx == 0, idx == len(js) - 1)))
                    fifo.append((1, mk_div(u)))
                fifo.append((212 if (h == HPC - 1 or not O_DMA_TR) else 1, mk_fin()))

            oh_sb = otsb.tile([128, HPC, CH], FP8, tag="oh", name=f"oh_sb{c}")
            ol_sb = otsb.tile([128, HPC, CH], FP8, tag="ol", name=f"ol_sb{c}")
            prev = None
            for h in range(HPC):
                gh = c * HPC + h
                drain_until(gh - 2)
                ps = emit_scores(h)
                if prev is not None:
                    push_pv(h - 1, prev, oh_sb, ol_sb)
                    fifo.append((0, lambda mid=gh - 1: markers_done.add(mid)))
                prev = ps
            push_pv(HPC - 1, prev, oh_sb, ol_sb)
            fifo.append((0, lambda mid=c * HPC + HPC - 1: markers_done.add(mid)))

            # ------------ output projection (queued as filler) ------------
            # One atomic thunk per dt_ tile: the st2 accumulation group must
            # open and close without another st2 alloc slipping in between
            # (the ring has 2 slots; a half-open group would deadlock).
            ystage = ystp.tile([128, NT, CH], BF, tag="yst", name=f"yst{c}")

            def mk_wo(dt_, c=c, oh_sb=oh_sb, ol_sb=ol_sb, ystage=ystage):
                def f():
                    yt_ps = ps_st2.tile([128, CH], F32, tag="st2",
                                        name=f"yt{c}_{dt_}")
                    i = 0
                    wo_terms = (((woh_sb, oh_sb), (wol_sb, oh_sb),
                                 (woh_sb, ol_sb)) if WO_OL_LAST else
                                terms(woh_sb, wol_sb, oh_sb, ol_sb))
                    for wt, ot in wo_terms:
                        for p in range(2):
                            nc.tensor.matmul(
                                yt_ps,
                                wt[:, 2 * p:2 * p + 2, dt_ * 128:(dt_ + 1) * 128],
                                ot[:, 2 * p:2 * p + 2, :],
                                start=(i == 0), stop=(i == 5),
                                perf_mode=DR, skip_group_check=True)
                            i += 1
                    if c == NCHUNK - 1 and dt_ % 2 == 0:
                        nc.scalar.copy(ystage[:, dt_, :], yt_ps)
                    else:
                        nc.vector.tensor_copy(ystage[:, dt_, :], yt_ps)
                return f

            def mk_store(i, w, c=c, ystage=ystage):
                def f():
                    q = nc.sync if c == NCHUNK - 1 else nc.gpsimd
                    q.dma_start(
                        yT_r[:, w * i:w * (i + 1), c * CH:(c + 1) * CH],
                        ystage[:, w * i:w * (i + 1), :])
                return f

            stw = 1 if c == NCHUNK - 1 else 4
            for dt_ in range(NT):
                ent = (641, mk_wo(dt_))
                defer = (c == 1 and dt_ >= NT - DEFER_WO_DTS)
                (deferred_wo if defer else fifo).append(ent)
                if dt_ % stw == stw - 1:
                    ent = (1, mk_store(dt_ // stw, stw))
                    (deferred_wo if defer else fifo).append(ent)
            if c == 2:
                # chunk 1's deferred wo lands behind wo(c2): extra PE filler
                # for chunk 3's Act-bound attention phase
                fifo.extend(deferred_wo)
                deferred_wo.clear()

        drain(1 << 30)

    nc.compile()
    return nc


def _host_prep(x, freqs_cos, freqs_sin, wq, wk, wv, wo):
    """Build the 8 per-core input maps (fp8 hi/lo splits + bf16 consts)."""
    cos_t = np.ascontiguousarray(freqs_cos.T)  # [64, S]
    sin_t = np.ascontiguousarray(freqs_sin.T)
    descale = 1.0 / (SX * SW)
    cs = np.repeat(cos_t, 2, axis=0).astype(np.float32) * descale  # [128, S]
    sn = np.repeat(sin_t, 2, axis=0).astype(np.float32) * descale
    sn[0::2] *= -1.0
    cs = cs.astype(NPBF)
    sn = sn.astype(NPBF)

    pswap = np.zeros((DH, DH), dtype=NPBF)
    idx = np.arange(0, DH, 2)
    pswap[idx, idx + 1] = 1.0
    pswap[idx + 1, idx] = 1.0

    ident = np.eye(128, dtype=NPBF)

    # maskd[jj, z] = 1 where z >= jj: the causal triangle of a diagonal
    # 128x128 block (same for every diagonal tile).
    z = np.arange(128)[None, :]
    jj = np.arange(128)[:, None]
    maskd = (z >= jj).astype(NPBF)

    def split(a, scale):
        s = (a * scale).astype(np.float32)
        hi = s.astype(NPFP8)
        lo = (s - hi.astype(np.float32)).astype(NPFP8)
        return hi, lo

    xTs = [split(np.ascontiguousarray(x[b].T), SX) for b in range(2)]
    wqs = split(wq, SW)
    wks = split(wk, SW)
    wvs = split(wv, SW)
    wos = split(wo, SW)

    in_maps = []
    for core in range(N_CORES):
        b, g = divmod(core, HPC)
        qs = slice(g * HPC * DH, (g + 1) * HPC * DH)
        ks = slice(g * DH, (g + 1) * DH)
        in_maps.append({
            "xh": xTs[b][0], "xl": xTs[b][1],
            "wqh": np.ascontiguousarray(wqs[0][:, qs]),
            "wql": np.ascontiguousarray(wqs[1][:, qs]),
            "wkh": np.ascontiguousarray(wks[0][:, ks]),
            "wkl": np.ascontiguousarray(wks[1][:, ks]),
            "wvh": np.ascontiguousarray(wvs[0][:, ks]),
            "wvl": np.ascontiguousarray(wvs[1][:, ks]),
            "woh": np.ascontiguousarray(wos[0][qs, :]),
            "wol": np.ascontiguousarray(wos[1][qs, :]),
            "cs": cs, "sn": sn, "pswap": pswap, "ident": ident, "maskd": maskd,
        })
    return in_maps


def kernel(x, freqs_cos, freqs_sin, mask, wq, wk, wv, wo):
    x = np.asarray(x, dtype=np.float32)
    freqs_cos = np.asarray(freqs_cos, dtype=np.float32)
    freqs_sin = np.asarray(freqs_sin, dtype=np.float32)
    wq = np.asarray(wq, dtype=np.float32)
    wk = np.asarray(wk, dtype=np.float32)
    wv = np.asarray(wv, dtype=np.float32)
    wo = np.asarray(wo, dtype=np.float32)

    if "nc" not in _CACHE:
        _CACHE["nc"] = _build()
    nc = _CACHE["nc"]

    in_maps = _host_prep(x, freqs_cos, freqs_sin, wq, wk, wv, wo)
    res = bass_utils.run_bass_kernel_spmd(nc, in_maps, core_ids=list(range(N_CORES)))

    out = np.empty((2, S, DM), dtype=np.float32)
    for b in range(2):
        acc = res.results[b * HPC]["yT"].astype(np.float32)
        for g in range(1, HPC):
            acc = acc + res.results[b * HPC + g]["yT"].astype(np.float32)
        out[b] = acc.T * Y_DESCALE
    return out
